# revision 38
# baseline (speedup 1.0000x reference)
"""Trainium2 Bass kernel for a GPT-style transformer block (no attn out-proj).

Sharding (8 cores): attention is tensor-parallel over heads -- core c handles
batch c//4 and heads [4*(c%4), 4*(c%4)+4) over the full 2048-token causal
sequence. The MLP is token-parallel (core c takes tokens [512*(c%4), ...+512)
of its batch). Attention outputs are exchanged with one bf16 AllGather inside
each 4-core batch group; each core picks its token columns with an indirect
DMA driven by a per-core index input (single static SPMD program).

v2 changes vs baseline: bf16 activations/weights (PSUM + LN row math stay
fp32), bf16 AllGather (half the wire bytes), host-side layouts making every
DMA contiguous per partition, full w_fc preload + streamed w_proj, exp
batched over 2 PSUM banks, Rsqrt for LN, reciprocal_approx_fast for softmax
denominators.
"""

import numpy as np

B, T, C = 2, 2048, 1024
H, D = 16, 64
HPC = 4          # heads per core
G = 4            # cores per batch group
TCH = T // G     # tokens per core for the MLP (512)
N_CORES = 8
EPS = 1e-5
FC = 4 * C
CO = C // 128    # 8
FO = FC // 128   # 32
NT = T // 512    # 4 token chunks
TT = T // 128    # 16 token tiles
DV = D + 1       # v rows incl denominator ones-row

_CACHE = {}
LAST_EXEC_NS = None
LAST_RESULTS = None


def _build():
    import concourse.tile as tile
    from concourse import bacc, mybir

    F32 = mybir.dt.float32
    BF16 = mybir.dt.bfloat16
    AF = mybir.ActivationFunctionType

    nc = bacc.Bacc("TRN2", target_bir_lowering=False, debug=False,
                   num_devices=N_CORES)

    def inp(name, shape, dt=BF16):
        return nc.dram_tensor(name, shape, dt, kind="ExternalInput").ap()

    x4 = inp("x4", [128, NT, CO, 512])
    x_res = inp("x_res", [128, CO, 512])
    w_qk = inp("w_qk", [128, CO, 512])
    b_qk = inp("b_qk", [128, 4], F32)
    w_v = inp("w_v", [128, CO, HPC * DV])     # v weights + zero ones-columns
    b_v = inp("b_v", [1, HPC * DV])           # v bias with 1.0 in ones-columns
    masks_i = inp("masks", [128, 4, 512])
    ones_col_i = inp("ones_col", [128, 1])
    ones_row_i = inp("ones_row", [1, 128])
    w_fc = inp("w_fc", [128, CO, FC])
    b_fc = inp("b_fc", [128, FO], F32)
    w_pj = inp("w_pj", [128, CO, FO, 128])
    b_pj = inp("b_pj", [128, CO], F32)
    g_idx = inp("g_idx", [128, CO], mybir.dt.int32)

    out_t = nc.dram_tensor("out_t", [128, CO, 512], BF16,
                           kind="ExternalOutput").ap()

    with nc.allow_low_precision(reason="bf16 kernel; tolerance is 2e-2"), \
         tile.TileContext(nc) as tc:
        # ---- persistent pools ----------------------------------------
        const_cm = tc.tile_pool(name="const", bufs=1)
        dram_cm = tc.tile_pool(name="dram", bufs=1, space="DRAM")
        wfc_cm = tc.tile_pool(name="wfc", bufs=1)
        qkv_cm = tc.tile_pool(name="qkv", bufs=1)
        const = const_cm.__enter__()
        dram = dram_cm.__enter__()
        wfc_pool = wfc_cm.__enter__()
        qkv_pool = qkv_cm.__enter__()

        ones_c = const.tile([128, 1], BF16)
        ones_r = const.tile([1, 128], BF16)
        mask_sb = const.tile([128, 4, 512], BF16)
        nc.sync.dma_start(ones_c[:], ones_col_i)
        nc.sync.dma_start(ones_r[:], ones_row_i)
        nc.sync.dma_start(mask_sb[:], masks_i)

        q_sb = qkv_pool.tile([128, 2, T], BF16)
        k_sb = qkv_pool.tile([128, 2, T], BF16)
        v_sb = qkv_pool.tile([128, TT, HPC * DV], BF16)   # [.,16,260]

        # w_fc preloaded in full during phases 1-2 (8 MB bf16)
        wfc_sb = wfc_pool.tile([128, CO, FC], BF16)
        nc.sync.dma_start(wfc_sb[:], w_fc)

        # two exchange pieces (heads 0-1, heads 2-3): the first AllGather
        # fires once heads 0-1 are done, overlapping its wire time with the
        # remaining heads' compute. Piece A covers even o-tiles of the
        # gathered features, piece B the odd ones, each across all 128
        # partitions.
        cc_in_a = dram.tile([2 * D, T], BF16)
        cc_in_b = dram.tile([2 * D, T], BF16)
        cc_out_a = dram.tile([G * 2 * D, T], BF16)
        cc_out_b = dram.tile([G * 2 * D, T], BF16)

        # ---- phase 1: load x, LN1 (in place), QKV --------------------
        with tc.tile_pool(name="xh", bufs=1) as xh_pool, \
             tc.tile_pool(name="ln_ps", bufs=1, space="PSUM") as ln_ps, \
             tc.tile_pool(name="ln_bc_ps", bufs=2, space="PSUM") as ln_bc_ps, \
             tc.tile_pool(name="ln_sb", bufs=1) as ln_sb, \
             tc.tile_pool(name="row", bufs=2) as row_pool, \
             tc.tile_pool(name="sq", bufs=3) as sq_pool, \
             tc.tile_pool(name="wq", bufs=1) as wq_pool, \
             tc.tile_pool(name="qk_ps", bufs=2, space="PSUM") as qk_ps, \
             tc.tile_pool(name="v_ps", bufs=2, space="PSUM") as v_ps:

            xh = xh_pool.tile([128, NT, CO, 512], BF16)
            for cn in range(NT):
                nc.sync.dma_start(xh[:, cn], x4[:, cn])

            wqk_sb = wq_pool.tile([128, CO, 512], BF16)
            bqk_sb = wq_pool.tile([128, 4], F32)
            wv_sb = wq_pool.tile([128, CO, HPC * DV], BF16)
            bv_sb = wq_pool.tile([1, HPC * DV], BF16)
            nc.sync.dma_start(wqk_sb[:], w_qk)
            nc.sync.dma_start(bqk_sb[:], b_qk)
            nc.sync.dma_start(wv_sb[:], w_v)
            nc.sync.dma_start(bv_sb[:], b_v)

            rstd_bc = ln_sb.tile([128, NT, 512], BF16)
            nm_bc = ln_sb.tile([128, NT, 512], BF16)
            inv_c = 1.0 / C
            for cn in range(NT):
                ps_s = ln_ps.tile([1, 512], F32, tag="ps_s")
                ps_q = ln_ps.tile([1, 512], F32, tag="ps_q")
                for o in range(CO):
                    sq = sq_pool.tile([128, 512], BF16)
                    nc.vector.tensor_mul(sq[:], xh[:, cn, o], xh[:, cn, o])
                    nc.tensor.matmul(ps_s[:], ones_c[:], xh[:, cn, o],
                                     start=(o == 0), stop=(o == CO - 1))
                    nc.tensor.matmul(ps_q[:], ones_c[:], sq[:],
                                     start=(o == 0), stop=(o == CO - 1))
                mu = row_pool.tile([1, 512], F32, tag="mu")
                msq = row_pool.tile([1, 512], F32, tag="msq")
                var = row_pool.tile([1, 512], F32, tag="var")
                std = row_pool.tile([1, 512], F32, tag="std")
                rstd = row_pool.tile([1, 512], F32, tag="rstd")
                nm = row_pool.tile([1, 512], F32, tag="nm")
                rstd_h = row_pool.tile([1, 512], BF16, tag="rstd_h")
                nm_h = row_pool.tile([1, 512], BF16, tag="nm_h")
                nc.vector.tensor_scalar_mul(mu[:], ps_s[:], inv_c)
                nc.vector.tensor_scalar_mul(msq[:], ps_q[:], inv_c)
                nc.vector.tensor_mul(var[:], mu[:], mu[:])
                nc.vector.tensor_tensor(var[:], msq[:], var[:],
                                        mybir.AluOpType.subtract)
                nc.vector.tensor_scalar_add(var[:], var[:], EPS)
                nc.scalar.activation(std[:], var[:], AF.Sqrt)
                nc.vector.reciprocal_approx_fast(rstd[:], std[:])
                nc.vector.tensor_mul(nm[:], mu[:], rstd[:])
                nc.vector.tensor_scalar_mul(nm[:], nm[:], -1.0)
                nc.vector.tensor_copy(rstd_h[:], rstd[:])
                nc.vector.tensor_copy(nm_h[:], nm[:])

                pb = ln_bc_ps.tile([128, 512], F32, tag="bc")
                nc.tensor.matmul(pb[:], ones_r[:], rstd_h[:],
                                 start=True, stop=True)
                nc.scalar.copy(rstd_bc[:, cn], pb[:])
                pb2 = ln_bc_ps.tile([128, 512], F32, tag="bc")
                nc.tensor.matmul(pb2[:], ones_r[:], nm_h[:],
                                 start=True, stop=True)
                nc.scalar.copy(nm_bc[:, cn], pb2[:])

                for o in range(CO):
                    nc.vector.tensor_mul(xh[:, cn, o], xh[:, cn, o],
                                         rstd_bc[:, cn])
                    nc.vector.tensor_add(xh[:, cn, o], xh[:, cn, o],
                                         nm_bc[:, cn])

                # --- q, k for this chunk ---
                sl = slice(cn * 512, cn * 512 + 512)
                for m in range(4):           # 2 q tiles then 2 k tiles
                    pq = qk_ps.tile([128, 512], F32, tag="mmps")
                    for o in range(CO):
                        nc.tensor.matmul(
                            pq[:], wqk_sb[:, o, m * 128:(m + 1) * 128],
                            xh[:, cn, o], start=(o == 0), stop=(o == CO - 1))
                    dest = q_sb[:, m, sl] if m < 2 else k_sb[:, m - 2, sl]
                    nc.scalar.activation(dest, pq[:], AF.Identity,
                                         bias=bqk_sb[:, m:m + 1])

                # --- v (token-major) for this chunk ---
                for lt in range(4):
                    tt = cn * 4 + lt
                    pv = v_ps.tile([128, HPC * DV], F32, tag="vps")
                    nc.tensor.matmul(pv[:], ones_r[:], bv_sb[:],
                                     start=True, stop=False)
                    for o in range(CO):
                        nc.tensor.matmul(
                            pv[:], xh[:, cn, o, lt * 128:(lt + 1) * 128],
                            wv_sb[:, o, :], start=False, stop=(o == CO - 1))
                    nc.vector.tensor_copy(v_sb[:, tt, :], pv[:])

        # ---- phase 2: attention -------------------------------------
        with tc.tile_pool(name="a", bufs=2) as a_pool, \
             tc.tile_pool(name="s_ps", bufs=2, space="PSUM") as s_ps, \
             tc.tile_pool(name="y_ps", bufs=2, space="PSUM") as y_ps, \
             tc.tile_pool(name="nb_ps", bufs=2, space="PSUM") as nb_ps, \
             tc.tile_pool(name="ysm", bufs=3) as ysm:
            for h in range(HPC):
                po = 64 * (h % 2)
                pt = h // 2
                for qb in range(NT):
                    qsl = slice(qb * 512, qb * 512 + 512)
                    nkv = 4 * qb + 4
                    a_t = a_pool.tile([128, TT, 512], BF16, tag="a")
                    for u in range(nkv // 2):
                        ps2 = s_ps.tile([128, 1024], F32, tag="s")
                        for j in range(2):
                            t = 2 * u + j
                            nc.tensor.matmul(
                                ps2[:, j * 512:(j + 1) * 512],
                                k_sb[po:po + 64, pt, t * 128:(t + 1) * 128],
                                q_sb[po:po + 64, pt, qsl],
                                start=True, stop=True)
                        dst = a_t[:, 2 * u:2 * u + 2, :] \
                            .rearrange("p a b -> p (a b)")
                        nc.scalar.activation(dst, ps2[:], AF.Exp, scale=0.125)
                        if 2 * u >= 4 * qb:      # diagonal pair: apply mask
                            mi = 2 * u - 4 * qb
                            nc.vector.tensor_mul(
                                a_t[:, 2 * u:2 * u + 2, :],
                                a_t[:, 2 * u:2 * u + 2, :],
                                mask_sb[:, mi:mi + 2, :])
                    py = y_ps.tile([DV, 512], F32)
                    for t in range(nkv):
                        nc.tensor.matmul(py[:],
                                         v_sb[:, t, h * DV:(h + 1) * DV],
                                         a_t[:, t, :],
                                         start=(t == 0), stop=(t == nkv - 1))
                    den = ysm.tile([1, 512], F32, tag="den")
                    rr = ysm.tile([1, 512], F32, tag="rr")
                    rr_h = ysm.tile([1, 512], BF16, tag="rr_h")
                    nc.vector.tensor_copy(den[:], py[D:D + 1, :])
                    nc.vector.reciprocal_approx_fast(rr[:], den[:])
                    nc.vector.tensor_copy(rr_h[:], rr[:])
                    pb = nb_ps.tile([D, 512], F32)
                    nc.tensor.matmul(pb[:], ones_r[:, 0:D], rr_h[:],
                                     start=True, stop=True)
                    rb = ysm.tile([D, 512], F32, tag="rb")
                    nc.vector.tensor_copy(rb[:], pb[:])
                    y_hqb = ysm.tile([D, 512], BF16, tag="y")
                    nc.vector.tensor_mul(y_hqb[:], py[0:D, :], rb[:])
                    piece = cc_in_a if h < 2 else cc_in_b
                    h2 = h % 2
                    nc.sync.dma_start(
                        piece[h2 * D:(h2 + 1) * D, qsl], y_hqb[:])

                if h == 1:
                    nc.gpsimd.collective_compute(
                        "AllGather", mybir.AluOpType.bypass,
                        replica_groups=[[0, 1, 2, 3], [4, 5, 6, 7]],
                        ins=[cc_in_a.opt()], outs=[cc_out_a.opt()])
                if h == 3:
                    nc.gpsimd.collective_compute(
                        "AllGather", mybir.AluOpType.bypass,
                        replica_groups=[[0, 1, 2, 3], [4, 5, 6, 7]],
                        ins=[cc_in_b.opt()], outs=[cc_out_b.opt()])

        qkv_cm.__exit__(None, None, None)

        # ---- phase 4: x2 = x + y, LN2, MLP --------------------------
        with tc.tile_pool(name="mlp", bufs=1) as mlp_pool, \
             tc.tile_pool(name="ln2_ps", bufs=1, space="PSUM") as ln2_ps, \
             tc.tile_pool(name="ln2_bc_ps", bufs=2, space="PSUM") as ln2_bc_ps, \
             tc.tile_pool(name="sq2", bufs=3) as sq2_pool, \
             tc.tile_pool(name="wp", bufs=3) as wp_pool, \
             tc.tile_pool(name="m_ps", bufs=4, space="PSUM") as m_ps, \
             tc.tile_pool(name="o_sb", bufs=3) as o_sb:

            import concourse.bass as bass_mod
            idx_sb = mlp_pool.tile([128, CO], mybir.dt.int32)
            nc.sync.dma_start(idx_sb[:], g_idx)
            xres_sb = mlp_pool.tile([128, CO, TCH], BF16)
            nc.sync.dma_start(xres_sb[:], x_res)
            y_sb = mlp_pool.tile([128, CO, TCH], BF16)
            tbl_a = cc_out_a[:].rearrange("c (blk t) -> (c blk) t", t=TCH)
            tbl_b = cc_out_b[:].rearrange("c (blk t) -> (c blk) t", t=TCH)
            x2 = mlp_pool.tile([128, CO, TCH], BF16)
            # even o-tiles draw from exchange piece A (heads 0-1), odd from
            # piece B -- process even tiles first so they flow while piece B
            # is still on the wire
            ps2_s = ln2_ps.tile([1, TCH], F32)
            ps2_q = ln2_ps.tile([1, TCH], F32)
            o_order = [0, 2, 4, 6, 1, 3, 5, 7]
            for oi, o in enumerate(o_order):
                nc.gpsimd.indirect_dma_start(
                    out=y_sb[:, o, :],
                    out_offset=None,
                    in_=(tbl_a if o % 2 == 0 else tbl_b),
                    in_offset=bass_mod.IndirectOffsetOnAxis(
                        ap=idx_sb[:, o:o + 1], axis=0),
                )
                nc.vector.tensor_add(x2[:, o], xres_sb[:, o], y_sb[:, o])
                sq = sq2_pool.tile([128, TCH], BF16)
                nc.vector.tensor_mul(sq[:], x2[:, o], x2[:, o])
                nc.tensor.matmul(ps2_s[:], ones_c[:], x2[:, o],
                                 start=(oi == 0), stop=(oi == CO - 1))
                nc.tensor.matmul(ps2_q[:], ones_c[:], sq[:],
                                 start=(oi == 0), stop=(oi == CO - 1))
            mu2 = mlp_pool.tile([1, TCH], F32)
            msq2 = mlp_pool.tile([1, TCH], F32)
            var2 = mlp_pool.tile([1, TCH], F32)
            std2 = mlp_pool.tile([1, TCH], F32)
            rstd2 = mlp_pool.tile([1, TCH], F32)
            nm2 = mlp_pool.tile([1, TCH], F32)
            rstd2_h = mlp_pool.tile([1, TCH], BF16)
            nm2_h = mlp_pool.tile([1, TCH], BF16)
            inv_c = 1.0 / C
            nc.vector.tensor_scalar_mul(mu2[:], ps2_s[:], inv_c)
            nc.vector.tensor_scalar_mul(msq2[:], ps2_q[:], inv_c)
            nc.vector.tensor_mul(var2[:], mu2[:], mu2[:])
            nc.vector.tensor_tensor(var2[:], msq2[:], var2[:],
                                    mybir.AluOpType.subtract)
            nc.vector.tensor_scalar_add(var2[:], var2[:], EPS)
            nc.scalar.activation(std2[:], var2[:], AF.Sqrt)
            nc.vector.reciprocal_approx_fast(rstd2[:], std2[:])
            nc.vector.tensor_mul(nm2[:], mu2[:], rstd2[:])
            nc.vector.tensor_scalar_mul(nm2[:], nm2[:], -1.0)
            nc.vector.tensor_copy(rstd2_h[:], rstd2[:])
            nc.vector.tensor_copy(nm2_h[:], nm2[:])

            pb = ln2_bc_ps.tile([128, TCH], F32, tag="bc2")
            nc.tensor.matmul(pb[:], ones_r[:], rstd2_h[:], start=True, stop=True)
            rstd2_bc = mlp_pool.tile([128, TCH], BF16)
            nc.scalar.copy(rstd2_bc[:], pb[:])
            pb2 = ln2_bc_ps.tile([128, TCH], F32, tag="bc2")
            nc.tensor.matmul(pb2[:], ones_r[:], nm2_h[:], start=True, stop=True)
            nm2_bc = mlp_pool.tile([128, TCH], BF16)
            nc.scalar.copy(nm2_bc[:], pb2[:])

            h2 = mlp_pool.tile([128, CO, TCH], BF16)
            for o in range(CO):
                nc.vector.tensor_mul(h2[:, o], x2[:, o], rstd2_bc[:])
                nc.vector.tensor_add(h2[:, o], h2[:, o], nm2_bc[:])

            # fc + gelu
            bfc_sb = mlp_pool.tile([128, FO], F32)
            nc.sync.dma_start(bfc_sb[:], b_fc)
            m_sb = mlp_pool.tile([128, FO, TCH], BF16)
            for mt in range(FO):
                pm = m_ps.tile([128, TCH], F32, tag="mm2")
                for o in range(CO):
                    nc.tensor.matmul(pm[:],
                                     wfc_sb[:, o, mt * 128:(mt + 1) * 128],
                                     h2[:, o], start=(o == 0),
                                     stop=(o == CO - 1))
                nc.scalar.activation(m_sb[:, mt], pm[:], AF.Gelu,
                                     bias=bfc_sb[:, mt:mt + 1])

            # proj + bias + residual (w_proj streamed per output tile)
            bpj_sb = mlp_pool.tile([128, CO], F32)
            nc.sync.dma_start(bpj_sb[:], b_pj)
            for o in range(CO):
                wt = wp_pool.tile([128, FO, 128], BF16, tag="wpj")
                nc.sync.dma_start(wt[:], w_pj[:, o])
                pp = m_ps.tile([128, TCH], F32, tag="mm2")
                for kt in range(FO):
                    nc.tensor.matmul(pp[:], wt[:, kt, :], m_sb[:, kt],
                                     start=(kt == 0), stop=(kt == FO - 1))
                po_sb = o_sb.tile([128, TCH], BF16, tag="po")
                nc.scalar.activation(po_sb[:], pp[:], AF.Identity,
                                     bias=bpj_sb[:, o:o + 1])
                fin = o_sb.tile([128, TCH], BF16, tag="fin")
                nc.vector.tensor_add(fin[:], po_sb[:], x2[:, o])
                nc.sync.dma_start(out_t[:, o], fin[:])

        for cm in (wfc_cm, dram_cm, const_cm):
            cm.__exit__(None, None, None)

    nc.compile()
    return nc


def _get_nc():
    if "nc" not in _CACHE:
        _CACHE["nc"] = _build()
    return _CACHE["nc"]


def _make_masks():
    # masks[p, t, j] = 1 if 128*t + p <= j else 0   (k-token vs q-token)
    m = np.zeros((128, 4, 512), np.float32)
    i = np.arange(128)[:, None]
    j = np.arange(512)[None, :]
    for t in range(4):
        m[:, t, :] = (128 * t + i <= j).astype(np.float32)
    return m


def kernel(x, ln1_g, ln1_b, W_attn, b_attn, ln2_g, ln2_b, W_fc, b_fc,
           W_proj, b_proj):
    global LAST_EXEC_NS, LAST_RESULTS
    import os
    import ml_dtypes

    from concourse.bass_utils import run_bass_kernel_spmd

    BF = ml_dtypes.bfloat16

    x = np.asarray(x, np.float32)
    W1 = np.asarray(ln1_g, np.float32)[:, None] * np.asarray(W_attn, np.float32)
    b1 = np.asarray(b_attn, np.float32) + np.asarray(ln1_b, np.float32) @ np.asarray(W_attn, np.float32)
    Wf = np.asarray(ln2_g, np.float32)[:, None] * np.asarray(W_fc, np.float32)
    bf = np.asarray(b_fc, np.float32) + np.asarray(ln2_b, np.float32) @ np.asarray(W_fc, np.float32)
    Wp = np.asarray(W_proj, np.float32)
    bp = np.asarray(b_proj, np.float32)

    masks = _make_masks().astype(BF)
    ones_col = np.ones((128, 1), BF)
    ones_row = np.ones((1, 128), BF)
    wfc_arr = np.ascontiguousarray(
        Wf.reshape(CO, 128, FC).transpose(1, 0, 2)).astype(BF)
    bfc_arr = np.ascontiguousarray(bf.reshape(FO, 128).T).astype(np.float32)
    wpj_arr = np.ascontiguousarray(
        Wp.reshape(FO, 128, CO, 128).transpose(1, 2, 0, 3)).astype(BF)
    bpj_arr = np.ascontiguousarray(bp.reshape(CO, 128).T).astype(np.float32)

    in_maps = []
    for c in range(N_CORES):
        b = c // G
        g = c % G
        tok0 = g * TCH
        qc = slice(g * HPC * D, (g + 1) * HPC * D)
        kc = slice(C + g * HPC * D, C + (g + 1) * HPC * D)
        vc = slice(2 * C + g * HPC * D, 2 * C + (g + 1) * HPC * D)
        xb = x[b]                                    # [T, C]
        x4_arr = np.ascontiguousarray(
            xb.reshape(NT, 512, CO, 128).transpose(3, 0, 2, 1)).astype(BF)
        xres_arr = np.ascontiguousarray(
            xb[tok0:tok0 + TCH].reshape(TCH, CO, 128).transpose(2, 1, 0)
        ).astype(BF)
        wqk = np.concatenate([W1[:, qc], W1[:, kc]], axis=1)   # [1024, 512]
        wqk_arr = np.ascontiguousarray(
            wqk.reshape(CO, 128, 512).transpose(1, 0, 2)).astype(BF)
        bqk = np.concatenate([b1[qc], b1[kc]])                  # [512]
        bqk_arr = np.ascontiguousarray(bqk.reshape(4, 128).T).astype(np.float32)
        # v weights augmented with a zero column per head whose bias is 1.0:
        # the V matmul then emits the softmax-denominator ones-row exactly.
        wv_aug = np.zeros((C, HPC * DV), np.float32)
        bv_aug = np.zeros((HPC * DV,), np.float32)
        for hh in range(HPC):
            wv_aug[:, hh * DV:hh * DV + D] = W1[:, vc][:, hh * D:(hh + 1) * D]
            bv_aug[hh * DV:hh * DV + D] = b1[vc][hh * D:(hh + 1) * D]
            bv_aug[hh * DV + D] = 1.0
        wv_arr = np.ascontiguousarray(
            wv_aug.reshape(CO, 128, HPC * DV).transpose(1, 0, 2)).astype(BF)
        bv_arr = np.ascontiguousarray(bv_aug[None, :]).astype(BF)
        pp_ = np.arange(128)[:, None]
        oo_ = np.arange(CO)[None, :]
        gidx_arr = np.ascontiguousarray(
            (4 * ((oo_ // 2) * 128 + pp_) + g).astype(np.int32))
        in_maps.append({
            "x4": x4_arr,
            "x_res": xres_arr,
            "g_idx": gidx_arr,
            "w_qk": wqk_arr,
            "b_qk": bqk_arr,
            "w_v": wv_arr,
            "b_v": bv_arr,
            "masks": masks,
            "ones_col": ones_col,
            "ones_row": ones_row,
            "w_fc": wfc_arr,
            "b_fc": bfc_arr,
            "w_pj": wpj_arr,
            "b_pj": bpj_arr,
        })

    nc = _get_nc()
    trace = os.environ.get("KERNEL_TRACE") == "1"
    kw = {}
    if trace:
        kw = dict(trace=True, trace_cores=list(range(N_CORES)))
    res = run_bass_kernel_spmd(nc, in_maps, core_ids=list(range(N_CORES)), **kw)
    LAST_EXEC_NS = res.exec_time_ns
    LAST_RESULTS = res

    out = np.empty((B, T, C), np.float32)
    for c in range(N_CORES):
        b = c // G
        tok0 = (c % G) * TCH
        o_arr = np.asarray(res.results[c]["out_t"]).astype(np.float32)
        out[b, tok0:tok0 + TCH, :] = o_arr.transpose(2, 1, 0).reshape(TCH, C)
    return out


# revision 40
# speedup vs baseline: 1.0687x; 1.0687x over previous
"""Trainium2 Bass kernel for a GPT-style transformer block (no attn out-proj).

Sharding (8 cores): attention is tensor-parallel over heads -- core c handles
batch c//4 and heads [4*(c%4), 4*(c%4)+4) over the full 2048-token causal
sequence. The MLP is token-parallel (core c takes tokens [512*(c%4), ...+512)
of its batch). Attention outputs are exchanged with one bf16 AllGather inside
each 4-core batch group; each core picks its token columns with an indirect
DMA driven by a per-core index input (single static SPMD program).

v2 changes vs baseline: bf16 activations/weights (PSUM + LN row math stay
fp32), bf16 AllGather (half the wire bytes), host-side layouts making every
DMA contiguous per partition, full w_fc preload + streamed w_proj, exp
batched over 2 PSUM banks, Rsqrt for LN, reciprocal_approx_fast for softmax
denominators.
"""

import numpy as np

B, T, C = 2, 2048, 1024
H, D = 16, 64
HPC = 4          # heads per core
G = 4            # cores per batch group
TCH = T // G     # tokens per core for the MLP (512)
N_CORES = 8
EPS = 1e-5
FC = 4 * C
CO = C // 128    # 8
FO = FC // 128   # 32
NT = T // 512    # 4 token chunks
TT = T // 128    # 16 token tiles
DV = D + 1       # v rows incl denominator ones-row

_CACHE = {}
LAST_EXEC_NS = None
LAST_RESULTS = None


def _build():
    import concourse.tile as tile
    from concourse import bacc, mybir

    F32 = mybir.dt.float32
    BF16 = mybir.dt.bfloat16
    AF = mybir.ActivationFunctionType

    nc = bacc.Bacc("TRN2", target_bir_lowering=False, debug=False,
                   num_devices=N_CORES)

    def inp(name, shape, dt=BF16):
        return nc.dram_tensor(name, shape, dt, kind="ExternalInput").ap()

    x4 = inp("x4", [128, NT, CO, 512])
    x_res = inp("x_res", [128, CO, 512])
    w_qk = inp("w_qk", [128, CO, 512])
    b_qk = inp("b_qk", [128, 4], F32)
    w_v = inp("w_v", [128, CO, HPC * DV])     # v weights + zero ones-columns
    b_v = inp("b_v", [1, HPC * DV])           # v bias with 1.0 in ones-columns
    masks_i = inp("masks", [128, 4, 512])
    ones_col_i = inp("ones_col", [128, 1])
    ones_row_i = inp("ones_row", [1, 128])
    w_fc = inp("w_fc", [128, CO, FC])
    b_fc = inp("b_fc", [128, FO], F32)
    w_pj = inp("w_pj", [128, CO, FO, 128])
    b_pj = inp("b_pj", [128, CO], F32)
    g_idx = inp("g_idx", [128, CO], mybir.dt.int32)

    out_t = nc.dram_tensor("out_t", [128, CO, 512], BF16,
                           kind="ExternalOutput").ap()

    with nc.allow_low_precision(reason="bf16 kernel; tolerance is 2e-2"), \
         tile.TileContext(nc) as tc:
        # ---- persistent pools ----------------------------------------
        const_cm = tc.tile_pool(name="const", bufs=1)
        dram_cm = tc.tile_pool(name="dram", bufs=1, space="DRAM")
        wfc_cm = tc.tile_pool(name="wfc", bufs=1)
        qkv_cm = tc.tile_pool(name="qkv", bufs=1)
        const = const_cm.__enter__()
        dram = dram_cm.__enter__()
        wfc_pool = wfc_cm.__enter__()
        qkv_pool = qkv_cm.__enter__()

        ones_c = const.tile([128, 1], BF16)
        ones_r = const.tile([1, 128], BF16)
        mask_sb = const.tile([128, 4, 512], BF16)
        nc.sync.dma_start(ones_c[:], ones_col_i)
        nc.sync.dma_start(ones_r[:], ones_row_i)
        nc.sync.dma_start(mask_sb[:], masks_i)

        q_sb = qkv_pool.tile([128, 2, T], BF16)
        k_sb = qkv_pool.tile([128, 2, T], BF16)
        v_sb = qkv_pool.tile([128, TT, HPC * DV], BF16)   # [.,16,260]

        # w_fc preloaded in full during phases 1-2 (8 MB bf16)
        wfc_sb = wfc_pool.tile([128, CO, FC], BF16)
        nc.sync.dma_start(wfc_sb[:], w_fc)

        # two exchange pieces (heads 0-1, heads 2-3): the first AllGather
        # fires once heads 0-1 are done, overlapping its wire time with the
        # remaining heads' compute. Piece A covers even o-tiles of the
        # gathered features, piece B the odd ones, each across all 128
        # partitions.
        cc_in_a = dram.tile([2 * D, T], BF16)
        cc_in_b = dram.tile([2 * D, T], BF16)
        cc_out_a = dram.tile([G * 2 * D, T], BF16)
        cc_out_b = dram.tile([G * 2 * D, T], BF16)

        # ---- phase 1: load x, LN1 (in place), QKV --------------------
        with tc.tile_pool(name="xh", bufs=1) as xh_pool, \
             tc.tile_pool(name="ln_ps", bufs=1, space="PSUM") as ln_ps, \
             tc.tile_pool(name="ln_bc_ps", bufs=2, space="PSUM") as ln_bc_ps, \
             tc.tile_pool(name="ln_sb", bufs=1) as ln_sb, \
             tc.tile_pool(name="row", bufs=2) as row_pool, \
             tc.tile_pool(name="sq", bufs=3) as sq_pool, \
             tc.tile_pool(name="wq", bufs=1) as wq_pool, \
             tc.tile_pool(name="qk_ps", bufs=2, space="PSUM") as qk_ps, \
             tc.tile_pool(name="v_ps", bufs=2, space="PSUM") as v_ps:

            xh = xh_pool.tile([128, NT, CO, 512], BF16)
            for cn in range(NT):
                nc.sync.dma_start(xh[:, cn], x4[:, cn])

            wqk_sb = wq_pool.tile([128, CO, 512], BF16)
            bqk_sb = wq_pool.tile([128, 4], F32)
            wv_sb = wq_pool.tile([128, CO, HPC * DV], BF16)
            bv_sb = wq_pool.tile([1, HPC * DV], BF16)
            nc.sync.dma_start(wqk_sb[:], w_qk)
            nc.sync.dma_start(bqk_sb[:], b_qk)
            nc.sync.dma_start(wv_sb[:], w_v)
            nc.sync.dma_start(bv_sb[:], b_v)

            rstd_bc = ln_sb.tile([128, NT, 512], BF16)
            nm_bc = ln_sb.tile([128, NT, 512], BF16)
            inv_c = 1.0 / C
            for cn in range(NT):
                ps_s = ln_ps.tile([1, 512], F32, tag="ps_s")
                ps_q = ln_ps.tile([1, 512], F32, tag="ps_q")
                for o in range(CO):
                    sq = sq_pool.tile([128, 512], BF16)
                    nc.vector.tensor_mul(sq[:], xh[:, cn, o], xh[:, cn, o])
                    nc.tensor.matmul(ps_s[:], ones_c[:], xh[:, cn, o],
                                     start=(o == 0), stop=(o == CO - 1))
                    nc.tensor.matmul(ps_q[:], ones_c[:], sq[:],
                                     start=(o == 0), stop=(o == CO - 1))
                mu = row_pool.tile([1, 512], F32, tag="mu")
                msq = row_pool.tile([1, 512], F32, tag="msq")
                var = row_pool.tile([1, 512], F32, tag="var")
                std = row_pool.tile([1, 512], F32, tag="std")
                rstd = row_pool.tile([1, 512], F32, tag="rstd")
                nm = row_pool.tile([1, 512], F32, tag="nm")
                rstd_h = row_pool.tile([1, 512], BF16, tag="rstd_h")
                nm_h = row_pool.tile([1, 512], BF16, tag="nm_h")
                nc.vector.tensor_scalar_mul(mu[:], ps_s[:], inv_c)
                nc.vector.tensor_scalar_mul(msq[:], ps_q[:], inv_c)
                nc.vector.tensor_mul(var[:], mu[:], mu[:])
                nc.vector.tensor_tensor(var[:], msq[:], var[:],
                                        mybir.AluOpType.subtract)
                nc.vector.tensor_scalar_add(var[:], var[:], EPS)
                nc.scalar.activation(std[:], var[:], AF.Sqrt)
                nc.vector.reciprocal_approx_fast(rstd[:], std[:])
                nc.vector.tensor_mul(nm[:], mu[:], rstd[:])
                nc.vector.tensor_scalar_mul(nm[:], nm[:], -1.0)
                nc.vector.tensor_copy(rstd_h[:], rstd[:])
                nc.vector.tensor_copy(nm_h[:], nm[:])

                pb = ln_bc_ps.tile([128, 512], F32, tag="bc")
                nc.tensor.matmul(pb[:], ones_r[:], rstd_h[:],
                                 start=True, stop=True)
                nc.scalar.copy(rstd_bc[:, cn], pb[:])
                pb2 = ln_bc_ps.tile([128, 512], F32, tag="bc")
                nc.tensor.matmul(pb2[:], ones_r[:], nm_h[:],
                                 start=True, stop=True)
                nc.scalar.copy(nm_bc[:, cn], pb2[:])

                for o in range(CO):
                    nc.vector.tensor_mul(xh[:, cn, o], xh[:, cn, o],
                                         rstd_bc[:, cn])
                    nc.vector.tensor_add(xh[:, cn, o], xh[:, cn, o],
                                         nm_bc[:, cn])

                # --- q, k for this chunk ---
                sl = slice(cn * 512, cn * 512 + 512)
                for m in range(4):           # 2 q tiles then 2 k tiles
                    pq = qk_ps.tile([128, 512], F32, tag="mmps")
                    for o in range(CO):
                        nc.tensor.matmul(
                            pq[:], wqk_sb[:, o, m * 128:(m + 1) * 128],
                            xh[:, cn, o], start=(o == 0), stop=(o == CO - 1))
                    dest = q_sb[:, m, sl] if m < 2 else k_sb[:, m - 2, sl]
                    nc.scalar.activation(dest, pq[:], AF.Identity,
                                         bias=bqk_sb[:, m:m + 1])

                # --- v (token-major) for this chunk ---
                for lt in range(4):
                    tt = cn * 4 + lt
                    pv = v_ps.tile([128, HPC * DV], F32, tag="vps")
                    nc.tensor.matmul(pv[:], ones_r[:], bv_sb[:],
                                     start=True, stop=False)
                    for o in range(CO):
                        nc.tensor.matmul(
                            pv[:], xh[:, cn, o, lt * 128:(lt + 1) * 128],
                            wv_sb[:, o, :], start=False, stop=(o == CO - 1))
                    nc.vector.tensor_copy(v_sb[:, tt, :], pv[:])

        # ---- phase 2: attention -------------------------------------
        with tc.tile_pool(name="a", bufs=2) as a_pool, \
             tc.tile_pool(name="s_ps", bufs=1, space="PSUM") as s_ps, \
             tc.tile_pool(name="y_ps", bufs=2, space="PSUM") as y_ps, \
             tc.tile_pool(name="nb_ps", bufs=2, space="PSUM") as nb_ps, \
             tc.tile_pool(name="ysm", bufs=3) as ysm:
            for pt in range(2):
                piece = cc_in_a if pt == 0 else cc_in_b
                for qb in range(NT):
                    qsl = slice(qb * 512, qb * 512 + 512)
                    nkv = 4 * qb + 4
                    a_tA = a_pool.tile([128, TT, 512], BF16, tag="aA")
                    a_tB = a_pool.tile([128, TT, 512], BF16, tag="aB")
                    for u in range(nkv // 2):
                        # the two heads' score matmuls occupy disjoint PE
                        # row-groups (K rows 0-63 vs 64-127) and different
                        # PSUM banks, so adjacent issue runs them
                        # concurrently in the array (row tiling)
                        psA = s_ps.tile([128, 1024], F32, tag="sA")
                        psB = s_ps.tile([128, 1024], F32, tag="sB")
                        for j in range(2):
                            t = 2 * u + j
                            jsl = slice(j * 512, (j + 1) * 512)
                            ksl = slice(t * 128, (t + 1) * 128)
                            nc.tensor.matmul(
                                psA[:, jsl], k_sb[0:64, pt, ksl],
                                q_sb[0:64, pt, qsl], start=True, stop=True)
                            nc.tensor.matmul(
                                psB[:, jsl], k_sb[64:128, pt, ksl],
                                q_sb[64:128, pt, qsl], start=True, stop=True)
                        for a_t, ps2 in ((a_tA, psA), (a_tB, psB)):
                            dst = a_t[:, 2 * u:2 * u + 2, :] \
                                .rearrange("p a b -> p (a b)")
                            nc.scalar.activation(dst, ps2[:], AF.Exp,
                                                 scale=0.125)
                            if 2 * u >= 4 * qb:   # diagonal pair: apply mask
                                mi = 2 * u - 4 * qb
                                nc.vector.tensor_mul(
                                    a_t[:, 2 * u:2 * u + 2, :],
                                    a_t[:, 2 * u:2 * u + 2, :],
                                    mask_sb[:, mi:mi + 2, :])
                    for h2, a_t in ((0, a_tA), (1, a_tB)):
                        h = 2 * pt + h2
                        py = y_ps.tile([DV, 512], F32, tag="y")
                        for t in range(nkv):
                            nc.tensor.matmul(py[:],
                                             v_sb[:, t, h * DV:(h + 1) * DV],
                                             a_t[:, t, :],
                                             start=(t == 0),
                                             stop=(t == nkv - 1))
                        den = ysm.tile([1, 512], F32, tag="den")
                        rr = ysm.tile([1, 512], F32, tag="rr")
                        rr_h = ysm.tile([1, 512], BF16, tag="rr_h")
                        nc.vector.tensor_copy(den[:], py[D:D + 1, :])
                        nc.vector.reciprocal_approx_fast(rr[:], den[:])
                        nc.vector.tensor_copy(rr_h[:], rr[:])
                        pb = nb_ps.tile([D, 512], F32)
                        nc.tensor.matmul(pb[:], ones_r[:, 0:D], rr_h[:],
                                         start=True, stop=True)
                        rb = ysm.tile([D, 512], F32, tag="rb")
                        nc.vector.tensor_copy(rb[:], pb[:])
                        y_hqb = ysm.tile([D, 512], BF16, tag="y2")
                        nc.vector.tensor_mul(y_hqb[:], py[0:D, :], rb[:])
                        nc.sync.dma_start(
                            piece[h2 * D:(h2 + 1) * D, qsl], y_hqb[:])

                nc.gpsimd.collective_compute(
                    "AllGather", mybir.AluOpType.bypass,
                    replica_groups=[[0, 1, 2, 3], [4, 5, 6, 7]],
                    ins=[(cc_in_a if pt == 0 else cc_in_b).opt()],
                    outs=[(cc_out_a if pt == 0 else cc_out_b).opt()])

        qkv_cm.__exit__(None, None, None)

        # ---- phase 4: x2 = x + y, LN2, MLP --------------------------
        with tc.tile_pool(name="mlp", bufs=1) as mlp_pool, \
             tc.tile_pool(name="ln2_ps", bufs=1, space="PSUM") as ln2_ps, \
             tc.tile_pool(name="ln2_bc_ps", bufs=2, space="PSUM") as ln2_bc_ps, \
             tc.tile_pool(name="sq2", bufs=3) as sq2_pool, \
             tc.tile_pool(name="wp", bufs=3) as wp_pool, \
             tc.tile_pool(name="m_ps", bufs=4, space="PSUM") as m_ps, \
             tc.tile_pool(name="o_sb", bufs=3) as o_sb:

            import concourse.bass as bass_mod
            idx_sb = mlp_pool.tile([128, CO], mybir.dt.int32)
            nc.sync.dma_start(idx_sb[:], g_idx)
            xres_sb = mlp_pool.tile([128, CO, TCH], BF16)
            nc.sync.dma_start(xres_sb[:], x_res)
            y_sb = mlp_pool.tile([128, CO, TCH], BF16)
            tbl_a = cc_out_a[:].rearrange("c (blk t) -> (c blk) t", t=TCH)
            tbl_b = cc_out_b[:].rearrange("c (blk t) -> (c blk) t", t=TCH)
            x2 = mlp_pool.tile([128, CO, TCH], BF16)
            # even o-tiles draw from exchange piece A (heads 0-1), odd from
            # piece B -- process even tiles first so they flow while piece B
            # is still on the wire
            ps2_s = ln2_ps.tile([1, TCH], F32)
            ps2_q = ln2_ps.tile([1, TCH], F32)
            o_order = [0, 2, 4, 6, 1, 3, 5, 7]
            for oi, o in enumerate(o_order):
                nc.gpsimd.indirect_dma_start(
                    out=y_sb[:, o, :],
                    out_offset=None,
                    in_=(tbl_a if o % 2 == 0 else tbl_b),
                    in_offset=bass_mod.IndirectOffsetOnAxis(
                        ap=idx_sb[:, o:o + 1], axis=0),
                )
                nc.vector.tensor_add(x2[:, o], xres_sb[:, o], y_sb[:, o])
                sq = sq2_pool.tile([128, TCH], BF16)
                nc.vector.tensor_mul(sq[:], x2[:, o], x2[:, o])
                nc.tensor.matmul(ps2_s[:], ones_c[:], x2[:, o],
                                 start=(oi == 0), stop=(oi == CO - 1))
                nc.tensor.matmul(ps2_q[:], ones_c[:], sq[:],
                                 start=(oi == 0), stop=(oi == CO - 1))
            mu2 = mlp_pool.tile([1, TCH], F32)
            msq2 = mlp_pool.tile([1, TCH], F32)
            var2 = mlp_pool.tile([1, TCH], F32)
            std2 = mlp_pool.tile([1, TCH], F32)
            rstd2 = mlp_pool.tile([1, TCH], F32)
            nm2 = mlp_pool.tile([1, TCH], F32)
            rstd2_h = mlp_pool.tile([1, TCH], BF16)
            nm2_h = mlp_pool.tile([1, TCH], BF16)
            inv_c = 1.0 / C
            nc.vector.tensor_scalar_mul(mu2[:], ps2_s[:], inv_c)
            nc.vector.tensor_scalar_mul(msq2[:], ps2_q[:], inv_c)
            nc.vector.tensor_mul(var2[:], mu2[:], mu2[:])
            nc.vector.tensor_tensor(var2[:], msq2[:], var2[:],
                                    mybir.AluOpType.subtract)
            nc.vector.tensor_scalar_add(var2[:], var2[:], EPS)
            nc.scalar.activation(std2[:], var2[:], AF.Sqrt)
            nc.vector.reciprocal_approx_fast(rstd2[:], std2[:])
            nc.vector.tensor_mul(nm2[:], mu2[:], rstd2[:])
            nc.vector.tensor_scalar_mul(nm2[:], nm2[:], -1.0)
            nc.vector.tensor_copy(rstd2_h[:], rstd2[:])
            nc.vector.tensor_copy(nm2_h[:], nm2[:])

            pb = ln2_bc_ps.tile([128, TCH], F32, tag="bc2")
            nc.tensor.matmul(pb[:], ones_r[:], rstd2_h[:], start=True, stop=True)
            rstd2_bc = mlp_pool.tile([128, TCH], BF16)
            nc.scalar.copy(rstd2_bc[:], pb[:])
            pb2 = ln2_bc_ps.tile([128, TCH], F32, tag="bc2")
            nc.tensor.matmul(pb2[:], ones_r[:], nm2_h[:], start=True, stop=True)
            nm2_bc = mlp_pool.tile([128, TCH], BF16)
            nc.scalar.copy(nm2_bc[:], pb2[:])

            h2 = mlp_pool.tile([128, CO, TCH], BF16)
            for o in range(CO):
                nc.vector.tensor_mul(h2[:, o], x2[:, o], rstd2_bc[:])
                nc.vector.tensor_add(h2[:, o], h2[:, o], nm2_bc[:])

            # fc + gelu
            bfc_sb = mlp_pool.tile([128, FO], F32)
            nc.sync.dma_start(bfc_sb[:], b_fc)
            m_sb = mlp_pool.tile([128, FO, TCH], BF16)
            for mt in range(FO):
                pm = m_ps.tile([128, TCH], F32, tag="mm2")
                for o in range(CO):
                    nc.tensor.matmul(pm[:],
                                     wfc_sb[:, o, mt * 128:(mt + 1) * 128],
                                     h2[:, o], start=(o == 0),
                                     stop=(o == CO - 1))
                nc.scalar.activation(m_sb[:, mt], pm[:], AF.Gelu,
                                     bias=bfc_sb[:, mt:mt + 1])

            # proj + bias + residual (w_proj streamed per output tile)
            bpj_sb = mlp_pool.tile([128, CO], F32)
            nc.sync.dma_start(bpj_sb[:], b_pj)
            for o in range(CO):
                wt = wp_pool.tile([128, FO, 128], BF16, tag="wpj")
                nc.sync.dma_start(wt[:], w_pj[:, o])
                pp = m_ps.tile([128, TCH], F32, tag="mm2")
                for kt in range(FO):
                    nc.tensor.matmul(pp[:], wt[:, kt, :], m_sb[:, kt],
                                     start=(kt == 0), stop=(kt == FO - 1))
                po_sb = o_sb.tile([128, TCH], BF16, tag="po")
                nc.scalar.activation(po_sb[:], pp[:], AF.Identity,
                                     bias=bpj_sb[:, o:o + 1])
                fin = o_sb.tile([128, TCH], BF16, tag="fin")
                nc.vector.tensor_add(fin[:], po_sb[:], x2[:, o])
                nc.sync.dma_start(out_t[:, o], fin[:])

        for cm in (wfc_cm, dram_cm, const_cm):
            cm.__exit__(None, None, None)

    nc.compile()
    return nc


def _get_nc():
    if "nc" not in _CACHE:
        _CACHE["nc"] = _build()
    return _CACHE["nc"]


def _make_masks():
    # masks[p, t, j] = 1 if 128*t + p <= j else 0   (k-token vs q-token)
    m = np.zeros((128, 4, 512), np.float32)
    i = np.arange(128)[:, None]
    j = np.arange(512)[None, :]
    for t in range(4):
        m[:, t, :] = (128 * t + i <= j).astype(np.float32)
    return m


def kernel(x, ln1_g, ln1_b, W_attn, b_attn, ln2_g, ln2_b, W_fc, b_fc,
           W_proj, b_proj):
    global LAST_EXEC_NS, LAST_RESULTS
    import os
    import ml_dtypes

    from concourse.bass_utils import run_bass_kernel_spmd

    BF = ml_dtypes.bfloat16

    x = np.asarray(x, np.float32)
    W1 = np.asarray(ln1_g, np.float32)[:, None] * np.asarray(W_attn, np.float32)
    b1 = np.asarray(b_attn, np.float32) + np.asarray(ln1_b, np.float32) @ np.asarray(W_attn, np.float32)
    Wf = np.asarray(ln2_g, np.float32)[:, None] * np.asarray(W_fc, np.float32)
    bf = np.asarray(b_fc, np.float32) + np.asarray(ln2_b, np.float32) @ np.asarray(W_fc, np.float32)
    Wp = np.asarray(W_proj, np.float32)
    bp = np.asarray(b_proj, np.float32)

    masks = _make_masks().astype(BF)
    ones_col = np.ones((128, 1), BF)
    ones_row = np.ones((1, 128), BF)
    wfc_arr = np.ascontiguousarray(
        Wf.reshape(CO, 128, FC).transpose(1, 0, 2)).astype(BF)
    bfc_arr = np.ascontiguousarray(bf.reshape(FO, 128).T).astype(np.float32)
    wpj_arr = np.ascontiguousarray(
        Wp.reshape(FO, 128, CO, 128).transpose(1, 2, 0, 3)).astype(BF)
    bpj_arr = np.ascontiguousarray(bp.reshape(CO, 128).T).astype(np.float32)

    in_maps = []
    for c in range(N_CORES):
        b = c // G
        g = c % G
        tok0 = g * TCH
        qc = slice(g * HPC * D, (g + 1) * HPC * D)
        kc = slice(C + g * HPC * D, C + (g + 1) * HPC * D)
        vc = slice(2 * C + g * HPC * D, 2 * C + (g + 1) * HPC * D)
        xb = x[b]                                    # [T, C]
        x4_arr = np.ascontiguousarray(
            xb.reshape(NT, 512, CO, 128).transpose(3, 0, 2, 1)).astype(BF)
        xres_arr = np.ascontiguousarray(
            xb[tok0:tok0 + TCH].reshape(TCH, CO, 128).transpose(2, 1, 0)
        ).astype(BF)
        wqk = np.concatenate([W1[:, qc], W1[:, kc]], axis=1)   # [1024, 512]
        wqk_arr = np.ascontiguousarray(
            wqk.reshape(CO, 128, 512).transpose(1, 0, 2)).astype(BF)
        bqk = np.concatenate([b1[qc], b1[kc]])                  # [512]
        bqk_arr = np.ascontiguousarray(bqk.reshape(4, 128).T).astype(np.float32)
        # v weights augmented with a zero column per head whose bias is 1.0:
        # the V matmul then emits the softmax-denominator ones-row exactly.
        wv_aug = np.zeros((C, HPC * DV), np.float32)
        bv_aug = np.zeros((HPC * DV,), np.float32)
        for hh in range(HPC):
            wv_aug[:, hh * DV:hh * DV + D] = W1[:, vc][:, hh * D:(hh + 1) * D]
            bv_aug[hh * DV:hh * DV + D] = b1[vc][hh * D:(hh + 1) * D]
            bv_aug[hh * DV + D] = 1.0
        wv_arr = np.ascontiguousarray(
            wv_aug.reshape(CO, 128, HPC * DV).transpose(1, 0, 2)).astype(BF)
        bv_arr = np.ascontiguousarray(bv_aug[None, :]).astype(BF)
        pp_ = np.arange(128)[:, None]
        oo_ = np.arange(CO)[None, :]
        gidx_arr = np.ascontiguousarray(
            (4 * ((oo_ // 2) * 128 + pp_) + g).astype(np.int32))
        in_maps.append({
            "x4": x4_arr,
            "x_res": xres_arr,
            "g_idx": gidx_arr,
            "w_qk": wqk_arr,
            "b_qk": bqk_arr,
            "w_v": wv_arr,
            "b_v": bv_arr,
            "masks": masks,
            "ones_col": ones_col,
            "ones_row": ones_row,
            "w_fc": wfc_arr,
            "b_fc": bfc_arr,
            "w_pj": wpj_arr,
            "b_pj": bpj_arr,
        })

    nc = _get_nc()
    trace = os.environ.get("KERNEL_TRACE") == "1"
    kw = {}
    if trace:
        kw = dict(trace=True, trace_cores=list(range(N_CORES)))
    res = run_bass_kernel_spmd(nc, in_maps, core_ids=list(range(N_CORES)), **kw)
    LAST_EXEC_NS = res.exec_time_ns
    LAST_RESULTS = res

    out = np.empty((B, T, C), np.float32)
    for c in range(N_CORES):
        b = c // G
        tok0 = (c % G) * TCH
        o_arr = np.asarray(res.results[c]["out_t"]).astype(np.float32)
        out[b, tok0:tok0 + TCH, :] = o_arr.transpose(2, 1, 0).reshape(TCH, C)
    return out


# revision 45
# speedup vs baseline: 1.1425x; 1.0691x over previous
"""Trainium2 Bass kernel for a GPT-style transformer block (no attn out-proj).

Sharding (8 cores): attention is tensor-parallel over heads -- core c handles
batch c//4 and heads [4*(c%4), 4*(c%4)+4) over the full 2048-token causal
sequence. The MLP is token-parallel (core c takes tokens [512*(c%4), ...+512)
of its batch). Attention outputs are exchanged with one bf16 AllGather inside
each 4-core batch group; each core picks its token columns with an indirect
DMA driven by a per-core index input (single static SPMD program).

v2 changes vs baseline: bf16 activations/weights (PSUM + LN row math stay
fp32), bf16 AllGather (half the wire bytes), host-side layouts making every
DMA contiguous per partition, full w_fc preload + streamed w_proj, exp
batched over 2 PSUM banks, Rsqrt for LN, reciprocal_approx_fast for softmax
denominators.
"""

import numpy as np

B, T, C = 2, 2048, 1024
H, D = 16, 64
HPC = 4          # heads per core
G = 4            # cores per batch group
TCH = T // G     # tokens per core for the MLP (512)
N_CORES = 8
EPS = 1e-5
FC = 4 * C
CO = C // 128    # 8
FO = FC // 128   # 32
NT = T // 512    # 4 token chunks
TT = T // 128    # 16 token tiles
DV = D + 1       # v rows incl denominator ones-row

_CACHE = {}
LAST_EXEC_NS = None
LAST_RESULTS = None


def _build():
    import concourse.tile as tile
    from concourse import bacc, mybir

    F32 = mybir.dt.float32
    BF16 = mybir.dt.bfloat16
    AF = mybir.ActivationFunctionType

    nc = bacc.Bacc("TRN2", target_bir_lowering=False, debug=False,
                   num_devices=N_CORES)

    def inp(name, shape, dt=BF16):
        return nc.dram_tensor(name, shape, dt, kind="ExternalInput").ap()

    x4 = inp("x4", [128, NT, CO, 512])
    x_res = inp("x_res", [128, CO, 512])
    w_qk = inp("w_qk", [128, CO, 512])
    b_qk = inp("b_qk", [128, 4], F32)
    w_v = inp("w_v", [128, CO, HPC * DV])     # v weights + zero ones-columns
    b_v = inp("b_v", [1, HPC * DV])           # v bias with 1.0 in ones-columns
    masks_i = inp("masks", [128, 4, 512])
    ones_col_i = inp("ones_col", [128, 1])
    ones_row_i = inp("ones_row", [1, 128])
    w_fc = inp("w_fc", [128, CO, FC])
    b_fc = inp("b_fc", [128, FO], F32)
    w_pj = inp("w_pj", [128, CO, FO, 128])
    b_pj = inp("b_pj", [128, CO], F32)
    g_idx = inp("g_idx", [128, CO], mybir.dt.int32)

    out_t = nc.dram_tensor("out_t", [128, CO, 512], BF16,
                           kind="ExternalOutput").ap()

    with nc.allow_low_precision(reason="bf16 kernel; tolerance is 2e-2"), \
         tile.TileContext(nc) as tc:
        # ---- persistent pools ----------------------------------------
        const_cm = tc.tile_pool(name="const", bufs=1)
        dram_cm = tc.tile_pool(name="dram", bufs=1, space="DRAM")
        wfc_cm = tc.tile_pool(name="wfc", bufs=1)
        qkv_cm = tc.tile_pool(name="qkv", bufs=1)
        const = const_cm.__enter__()
        dram = dram_cm.__enter__()
        wfc_pool = wfc_cm.__enter__()
        qkv_pool = qkv_cm.__enter__()

        ones_c = const.tile([128, 1], BF16)
        ones_r = const.tile([1, 128], BF16)
        mask_sb = const.tile([128, 4, 512], BF16)
        nc.sync.dma_start(ones_c[:], ones_col_i)
        nc.sync.dma_start(ones_r[:], ones_row_i)
        nc.sync.dma_start(mask_sb[:], masks_i)

        q_sb = qkv_pool.tile([128, 2, T], BF16)
        k_sb = qkv_pool.tile([128, 2, T], BF16)
        v_sb = qkv_pool.tile([128, TT, HPC * DV], BF16)   # [.,16,260]

        # w_fc preloaded in full during phases 1-2 (8 MB bf16)
        wfc_sb = wfc_pool.tile([128, CO, FC], BF16)
        nc.sync.dma_start(wfc_sb[:], w_fc)

        # one exchange piece per 512-token chunk: AllGather #qb fires as soon
        # as all four local heads have finished chunk qb, so pieces 0-2 hide
        # entirely under the remaining attention compute and only the last
        # (smallest-wire) piece is exposed. All pieces gather into slices of
        # one output tile so a single per-core index table can select the
        # local chunk.
        cc_in_q = [dram.tile([HPC * D, TCH], BF16, name=f"cc_in_q{i}")
                   for i in range(NT)]
        cc_out_all = dram.tile([NT, G * HPC * D, TCH], BF16)

        # ---- phase 1: load x, LN1 (in place), QKV --------------------
        with tc.tile_pool(name="xh", bufs=1) as xh_pool, \
             tc.tile_pool(name="ln_ps", bufs=1, space="PSUM") as ln_ps, \
             tc.tile_pool(name="ln_bc_ps", bufs=2, space="PSUM") as ln_bc_ps, \
             tc.tile_pool(name="ln_sb", bufs=1) as ln_sb, \
             tc.tile_pool(name="row", bufs=2) as row_pool, \
             tc.tile_pool(name="sq", bufs=3) as sq_pool, \
             tc.tile_pool(name="wq", bufs=1) as wq_pool, \
             tc.tile_pool(name="qk_ps", bufs=2, space="PSUM") as qk_ps, \
             tc.tile_pool(name="v_ps", bufs=2, space="PSUM") as v_ps:

            xh = xh_pool.tile([128, NT, CO, 512], BF16)
            for cn in range(NT):
                nc.sync.dma_start(xh[:, cn], x4[:, cn])

            wqk_sb = wq_pool.tile([128, CO, 512], BF16)
            bqk_sb = wq_pool.tile([128, 4], F32)
            wv_sb = wq_pool.tile([128, CO, HPC * DV], BF16)
            bv_sb = wq_pool.tile([1, HPC * DV], BF16)
            nc.sync.dma_start(wqk_sb[:], w_qk)
            nc.sync.dma_start(bqk_sb[:], b_qk)
            nc.sync.dma_start(wv_sb[:], w_v)
            nc.sync.dma_start(bv_sb[:], b_v)

            rstd_bc = ln_sb.tile([128, NT, 512], BF16)
            nm_bc = ln_sb.tile([128, NT, 512], BF16)
            inv_c = 1.0 / C
            for cn in range(NT):
                ps_s = ln_ps.tile([1, 512], F32, tag="ps_s")
                ps_q = ln_ps.tile([1, 512], F32, tag="ps_q")
                for o in range(CO):
                    sq = sq_pool.tile([128, 512], BF16)
                    nc.vector.tensor_mul(sq[:], xh[:, cn, o], xh[:, cn, o])
                    nc.tensor.matmul(ps_s[:], ones_c[:], xh[:, cn, o],
                                     start=(o == 0), stop=(o == CO - 1))
                    nc.tensor.matmul(ps_q[:], ones_c[:], sq[:],
                                     start=(o == 0), stop=(o == CO - 1))
                mu = row_pool.tile([1, 512], F32, tag="mu")
                msq = row_pool.tile([1, 512], F32, tag="msq")
                var = row_pool.tile([1, 512], F32, tag="var")
                std = row_pool.tile([1, 512], F32, tag="std")
                rstd = row_pool.tile([1, 512], F32, tag="rstd")
                nm = row_pool.tile([1, 512], F32, tag="nm")
                rstd_h = row_pool.tile([1, 512], BF16, tag="rstd_h")
                nm_h = row_pool.tile([1, 512], BF16, tag="nm_h")
                nc.vector.tensor_scalar_mul(mu[:], ps_s[:], inv_c)
                nc.vector.tensor_scalar_mul(msq[:], ps_q[:], inv_c)
                nc.vector.tensor_mul(var[:], mu[:], mu[:])
                nc.vector.tensor_tensor(var[:], msq[:], var[:],
                                        mybir.AluOpType.subtract)
                nc.vector.tensor_scalar_add(var[:], var[:], EPS)
                nc.scalar.activation(std[:], var[:], AF.Sqrt)
                nc.vector.reciprocal_approx_fast(rstd[:], std[:])
                nc.vector.tensor_mul(nm[:], mu[:], rstd[:])
                nc.vector.tensor_scalar_mul(nm[:], nm[:], -1.0)
                nc.vector.tensor_copy(rstd_h[:], rstd[:])
                nc.vector.tensor_copy(nm_h[:], nm[:])

                pb = ln_bc_ps.tile([128, 512], F32, tag="bc")
                nc.tensor.matmul(pb[:], ones_r[:], rstd_h[:],
                                 start=True, stop=True)
                nc.scalar.copy(rstd_bc[:, cn], pb[:])
                pb2 = ln_bc_ps.tile([128, 512], F32, tag="bc")
                nc.tensor.matmul(pb2[:], ones_r[:], nm_h[:],
                                 start=True, stop=True)
                nc.scalar.copy(nm_bc[:, cn], pb2[:])

                for o in range(CO):
                    nc.vector.tensor_mul(xh[:, cn, o], xh[:, cn, o],
                                         rstd_bc[:, cn])
                    nc.vector.tensor_add(xh[:, cn, o], xh[:, cn, o],
                                         nm_bc[:, cn])

                # --- q, k for this chunk ---
                sl = slice(cn * 512, cn * 512 + 512)
                for m in range(4):           # 2 q tiles then 2 k tiles
                    pq = qk_ps.tile([128, 512], F32, tag="mmps")
                    for o in range(CO):
                        nc.tensor.matmul(
                            pq[:], wqk_sb[:, o, m * 128:(m + 1) * 128],
                            xh[:, cn, o], start=(o == 0), stop=(o == CO - 1))
                    dest = q_sb[:, m, sl] if m < 2 else k_sb[:, m - 2, sl]
                    nc.scalar.activation(dest, pq[:], AF.Identity,
                                         bias=bqk_sb[:, m:m + 1])

                # --- v (token-major) for this chunk ---
                for lt in range(4):
                    tt = cn * 4 + lt
                    pv = v_ps.tile([128, HPC * DV], F32, tag="vps")
                    nc.tensor.matmul(pv[:], ones_r[:], bv_sb[:],
                                     start=True, stop=False)
                    for o in range(CO):
                        nc.tensor.matmul(
                            pv[:], xh[:, cn, o, lt * 128:(lt + 1) * 128],
                            wv_sb[:, o, :], start=False, stop=(o == CO - 1))
                    nc.vector.tensor_copy(v_sb[:, tt, :], pv[:])

        # ---- phase 2: attention -------------------------------------
        with tc.tile_pool(name="a", bufs=2) as a_pool, \
             tc.tile_pool(name="s_ps", bufs=1, space="PSUM") as s_ps, \
             tc.tile_pool(name="y_ps", bufs=2, space="PSUM") as y_ps, \
             tc.tile_pool(name="nb_ps", bufs=2, space="PSUM") as nb_ps, \
             tc.tile_pool(name="ysm", bufs=3) as ysm:
            for qb in range(NT):
                for pt in range(2):
                    qsl = slice(qb * 512, qb * 512 + 512)
                    nkv = 4 * qb + 4
                    a_tA = a_pool.tile([128, TT, 512], BF16, tag="aA")
                    a_tB = a_pool.tile([128, TT, 512], BF16, tag="aB")
                    for u in range(nkv // 2):
                        # the two heads' score matmuls occupy disjoint PE
                        # row-groups (K rows 0-63 vs 64-127) and different
                        # PSUM banks, so adjacent issue runs them
                        # concurrently in the array (row tiling)
                        psA = s_ps.tile([128, 1024], F32, tag="sA")
                        psB = s_ps.tile([128, 1024], F32, tag="sB")
                        for j in range(2):
                            t = 2 * u + j
                            jsl = slice(j * 512, (j + 1) * 512)
                            ksl = slice(t * 128, (t + 1) * 128)
                            nc.tensor.matmul(
                                psA[:, jsl], k_sb[0:64, pt, ksl],
                                q_sb[0:64, pt, qsl], start=True, stop=True)
                            nc.tensor.matmul(
                                psB[:, jsl], k_sb[64:128, pt, ksl],
                                q_sb[64:128, pt, qsl], start=True, stop=True)
                        for a_t, ps2 in ((a_tA, psA), (a_tB, psB)):
                            dst = a_t[:, 2 * u:2 * u + 2, :] \
                                .rearrange("p a b -> p (a b)")
                            nc.scalar.activation(dst, ps2[:], AF.Exp,
                                                 scale=0.125)
                            if 2 * u >= 4 * qb:   # diagonal pair: apply mask
                                mi = 2 * u - 4 * qb
                                nc.vector.tensor_mul(
                                    a_t[:, 2 * u:2 * u + 2, :],
                                    a_t[:, 2 * u:2 * u + 2, :],
                                    mask_sb[:, mi:mi + 2, :])
                    for h2, a_t in ((0, a_tA), (1, a_tB)):
                        h = 2 * pt + h2
                        py = y_ps.tile([DV, 512], F32, tag="y")
                        for t in range(nkv):
                            nc.tensor.matmul(py[:],
                                             v_sb[:, t, h * DV:(h + 1) * DV],
                                             a_t[:, t, :],
                                             start=(t == 0),
                                             stop=(t == nkv - 1))
                        den = ysm.tile([1, 512], F32, tag="den")
                        rr = ysm.tile([1, 512], F32, tag="rr")
                        rr_h = ysm.tile([1, 512], BF16, tag="rr_h")
                        nc.vector.tensor_copy(den[:], py[D:D + 1, :])
                        nc.vector.reciprocal_approx_fast(rr[:], den[:])
                        nc.vector.tensor_copy(rr_h[:], rr[:])
                        pb = nb_ps.tile([D, 512], F32)
                        nc.tensor.matmul(pb[:], ones_r[:, 0:D], rr_h[:],
                                         start=True, stop=True)
                        rb = ysm.tile([D, 512], F32, tag="rb")
                        nc.vector.tensor_copy(rb[:], pb[:])
                        y_hqb = ysm.tile([D, 512], BF16, tag="y2")
                        nc.vector.tensor_mul(y_hqb[:], py[0:D, :], rb[:])
                        nc.sync.dma_start(
                            cc_in_q[qb][h * D:(h + 1) * D, :], y_hqb[:])

                nc.gpsimd.collective_compute(
                    "AllGather", mybir.AluOpType.bypass,
                    replica_groups=[[0, 1, 2, 3], [4, 5, 6, 7]],
                    ins=[cc_in_q[qb].opt()],
                    outs=[cc_out_all[qb].opt()])

        qkv_cm.__exit__(None, None, None)

        # ---- phase 4: x2 = x + y, LN2, MLP --------------------------
        with tc.tile_pool(name="mlp", bufs=1) as mlp_pool, \
             tc.tile_pool(name="ln2_ps", bufs=1, space="PSUM") as ln2_ps, \
             tc.tile_pool(name="ln2_bc_ps", bufs=2, space="PSUM") as ln2_bc_ps, \
             tc.tile_pool(name="sq2", bufs=3) as sq2_pool, \
             tc.tile_pool(name="wp", bufs=3) as wp_pool, \
             tc.tile_pool(name="m_ps", bufs=4, space="PSUM") as m_ps, \
             tc.tile_pool(name="o_sb", bufs=3) as o_sb:

            import concourse.bass as bass_mod
            idx_sb = mlp_pool.tile([128, CO], mybir.dt.int32)
            nc.sync.dma_start(idx_sb[:], g_idx)
            xres_sb = mlp_pool.tile([128, CO, TCH], BF16)
            nc.sync.dma_start(xres_sb[:], x_res)
            y_sb = mlp_pool.tile([128, CO, TCH], BF16)
            tbl = cc_out_all[:].rearrange("q c t -> (q c) t")
            x2 = mlp_pool.tile([128, CO, TCH], BF16)
            ps2_s = ln2_ps.tile([1, TCH], F32)
            ps2_q = ln2_ps.tile([1, TCH], F32)
            for o in range(CO):
                nc.gpsimd.indirect_dma_start(
                    out=y_sb[:, o, :],
                    out_offset=None,
                    in_=tbl,
                    in_offset=bass_mod.IndirectOffsetOnAxis(
                        ap=idx_sb[:, o:o + 1], axis=0),
                )
                nc.vector.tensor_add(x2[:, o], xres_sb[:, o], y_sb[:, o])
                sq = sq2_pool.tile([128, TCH], BF16)
                nc.vector.tensor_mul(sq[:], x2[:, o], x2[:, o])
                nc.tensor.matmul(ps2_s[:], ones_c[:], x2[:, o],
                                 start=(o == 0), stop=(o == CO - 1))
                nc.tensor.matmul(ps2_q[:], ones_c[:], sq[:],
                                 start=(o == 0), stop=(o == CO - 1))
            mu2 = mlp_pool.tile([1, TCH], F32)
            msq2 = mlp_pool.tile([1, TCH], F32)
            var2 = mlp_pool.tile([1, TCH], F32)
            std2 = mlp_pool.tile([1, TCH], F32)
            rstd2 = mlp_pool.tile([1, TCH], F32)
            nm2 = mlp_pool.tile([1, TCH], F32)
            rstd2_h = mlp_pool.tile([1, TCH], BF16)
            nm2_h = mlp_pool.tile([1, TCH], BF16)
            inv_c = 1.0 / C
            nc.vector.tensor_scalar_mul(mu2[:], ps2_s[:], inv_c)
            nc.vector.tensor_scalar_mul(msq2[:], ps2_q[:], inv_c)
            nc.vector.tensor_mul(var2[:], mu2[:], mu2[:])
            nc.vector.tensor_tensor(var2[:], msq2[:], var2[:],
                                    mybir.AluOpType.subtract)
            nc.vector.tensor_scalar_add(var2[:], var2[:], EPS)
            nc.scalar.activation(std2[:], var2[:], AF.Sqrt)
            nc.vector.reciprocal_approx_fast(rstd2[:], std2[:])
            nc.vector.tensor_mul(nm2[:], mu2[:], rstd2[:])
            nc.vector.tensor_scalar_mul(nm2[:], nm2[:], -1.0)
            nc.vector.tensor_copy(rstd2_h[:], rstd2[:])
            nc.vector.tensor_copy(nm2_h[:], nm2[:])

            pb = ln2_bc_ps.tile([128, TCH], F32, tag="bc2")
            nc.tensor.matmul(pb[:], ones_r[:], rstd2_h[:], start=True, stop=True)
            rstd2_bc = mlp_pool.tile([128, TCH], BF16)
            nc.scalar.copy(rstd2_bc[:], pb[:])
            pb2 = ln2_bc_ps.tile([128, TCH], F32, tag="bc2")
            nc.tensor.matmul(pb2[:], ones_r[:], nm2_h[:], start=True, stop=True)
            nm2_bc = mlp_pool.tile([128, TCH], BF16)
            nc.scalar.copy(nm2_bc[:], pb2[:])

            h2 = mlp_pool.tile([128, CO, TCH], BF16)
            for o in range(CO):
                nc.vector.tensor_mul(h2[:, o], x2[:, o], rstd2_bc[:])
                nc.vector.tensor_add(h2[:, o], h2[:, o], nm2_bc[:])

            # fc + gelu
            bfc_sb = mlp_pool.tile([128, FO], F32)
            nc.sync.dma_start(bfc_sb[:], b_fc)
            m_sb = mlp_pool.tile([128, FO, TCH], BF16)
            for mt in range(FO):
                pm = m_ps.tile([128, TCH], F32, tag="mm2")
                for o in range(CO):
                    nc.tensor.matmul(pm[:],
                                     wfc_sb[:, o, mt * 128:(mt + 1) * 128],
                                     h2[:, o], start=(o == 0),
                                     stop=(o == CO - 1))
                nc.scalar.activation(m_sb[:, mt], pm[:], AF.Gelu,
                                     bias=bfc_sb[:, mt:mt + 1])

            # proj + bias + residual (w_proj streamed per output tile)
            bpj_sb = mlp_pool.tile([128, CO], F32)
            nc.sync.dma_start(bpj_sb[:], b_pj)
            for o in range(CO):
                wt = wp_pool.tile([128, FO, 128], BF16, tag="wpj")
                nc.sync.dma_start(wt[:], w_pj[:, o])
                pp = m_ps.tile([128, TCH], F32, tag="mm2")
                for kt in range(FO):
                    nc.tensor.matmul(pp[:], wt[:, kt, :], m_sb[:, kt],
                                     start=(kt == 0), stop=(kt == FO - 1))
                po_sb = o_sb.tile([128, TCH], BF16, tag="po")
                nc.scalar.activation(po_sb[:], pp[:], AF.Identity,
                                     bias=bpj_sb[:, o:o + 1])
                fin = o_sb.tile([128, TCH], BF16, tag="fin")
                nc.vector.tensor_add(fin[:], po_sb[:], x2[:, o])
                nc.sync.dma_start(out_t[:, o], fin[:])

        for cm in (wfc_cm, dram_cm, const_cm):
            cm.__exit__(None, None, None)

    nc.compile()
    return nc


def _get_nc():
    if "nc" not in _CACHE:
        _CACHE["nc"] = _build()
    return _CACHE["nc"]


def _make_masks():
    # masks[p, t, j] = 1 if 128*t + p <= j else 0   (k-token vs q-token)
    m = np.zeros((128, 4, 512), np.float32)
    i = np.arange(128)[:, None]
    j = np.arange(512)[None, :]
    for t in range(4):
        m[:, t, :] = (128 * t + i <= j).astype(np.float32)
    return m


def kernel(x, ln1_g, ln1_b, W_attn, b_attn, ln2_g, ln2_b, W_fc, b_fc,
           W_proj, b_proj):
    global LAST_EXEC_NS, LAST_RESULTS
    import os
    import ml_dtypes

    from concourse.bass_utils import run_bass_kernel_spmd

    BF = ml_dtypes.bfloat16

    x = np.asarray(x, np.float32)
    W1 = np.asarray(ln1_g, np.float32)[:, None] * np.asarray(W_attn, np.float32)
    b1 = np.asarray(b_attn, np.float32) + np.asarray(ln1_b, np.float32) @ np.asarray(W_attn, np.float32)
    Wf = np.asarray(ln2_g, np.float32)[:, None] * np.asarray(W_fc, np.float32)
    bf = np.asarray(b_fc, np.float32) + np.asarray(ln2_b, np.float32) @ np.asarray(W_fc, np.float32)
    Wp = np.asarray(W_proj, np.float32)
    bp = np.asarray(b_proj, np.float32)

    masks = _make_masks().astype(BF)
    ones_col = np.ones((128, 1), BF)
    ones_row = np.ones((1, 128), BF)
    wfc_arr = np.ascontiguousarray(
        Wf.reshape(CO, 128, FC).transpose(1, 0, 2)).astype(BF)
    bfc_arr = np.ascontiguousarray(bf.reshape(FO, 128).T).astype(np.float32)
    wpj_arr = np.ascontiguousarray(
        Wp.reshape(FO, 128, CO, 128).transpose(1, 2, 0, 3)).astype(BF)
    bpj_arr = np.ascontiguousarray(bp.reshape(CO, 128).T).astype(np.float32)

    in_maps = []
    for c in range(N_CORES):
        b = c // G
        g = c % G
        tok0 = g * TCH
        qc = slice(g * HPC * D, (g + 1) * HPC * D)
        kc = slice(C + g * HPC * D, C + (g + 1) * HPC * D)
        vc = slice(2 * C + g * HPC * D, 2 * C + (g + 1) * HPC * D)
        xb = x[b]                                    # [T, C]
        x4_arr = np.ascontiguousarray(
            xb.reshape(NT, 512, CO, 128).transpose(3, 0, 2, 1)).astype(BF)
        xres_arr = np.ascontiguousarray(
            xb[tok0:tok0 + TCH].reshape(TCH, CO, 128).transpose(2, 1, 0)
        ).astype(BF)
        wqk = np.concatenate([W1[:, qc], W1[:, kc]], axis=1)   # [1024, 512]
        wqk_arr = np.ascontiguousarray(
            wqk.reshape(CO, 128, 512).transpose(1, 0, 2)).astype(BF)
        bqk = np.concatenate([b1[qc], b1[kc]])                  # [512]
        bqk_arr = np.ascontiguousarray(bqk.reshape(4, 128).T).astype(np.float32)
        # v weights augmented with a zero column per head whose bias is 1.0:
        # the V matmul then emits the softmax-denominator ones-row exactly.
        wv_aug = np.zeros((C, HPC * DV), np.float32)
        bv_aug = np.zeros((HPC * DV,), np.float32)
        for hh in range(HPC):
            wv_aug[:, hh * DV:hh * DV + D] = W1[:, vc][:, hh * D:(hh + 1) * D]
            bv_aug[hh * DV:hh * DV + D] = b1[vc][hh * D:(hh + 1) * D]
            bv_aug[hh * DV + D] = 1.0
        wv_arr = np.ascontiguousarray(
            wv_aug.reshape(CO, 128, HPC * DV).transpose(1, 0, 2)).astype(BF)
        bv_arr = np.ascontiguousarray(bv_aug[None, :]).astype(BF)
        pp_ = np.arange(128)[:, None]
        oo_ = np.arange(CO)[None, :]
        gidx_arr = np.ascontiguousarray(
            (g * C + oo_ * 128 + pp_).astype(np.int32))
        in_maps.append({
            "x4": x4_arr,
            "x_res": xres_arr,
            "g_idx": gidx_arr,
            "w_qk": wqk_arr,
            "b_qk": bqk_arr,
            "w_v": wv_arr,
            "b_v": bv_arr,
            "masks": masks,
            "ones_col": ones_col,
            "ones_row": ones_row,
            "w_fc": wfc_arr,
            "b_fc": bfc_arr,
            "w_pj": wpj_arr,
            "b_pj": bpj_arr,
        })

    nc = _get_nc()
    trace = os.environ.get("KERNEL_TRACE") == "1"
    kw = {}
    if trace:
        kw = dict(trace=True, trace_cores=list(range(N_CORES)))
    res = run_bass_kernel_spmd(nc, in_maps, core_ids=list(range(N_CORES)), **kw)
    LAST_EXEC_NS = res.exec_time_ns
    LAST_RESULTS = res

    out = np.empty((B, T, C), np.float32)
    for c in range(N_CORES):
        b = c // G
        tok0 = (c % G) * TCH
        o_arr = np.asarray(res.results[c]["out_t"]).astype(np.float32)
        out[b, tok0:tok0 + TCH, :] = o_arr.transpose(2, 1, 0).reshape(TCH, C)
    return out
